# revision 15
# baseline (speedup 1.0000x reference)
"""CrossBatchAttention Trainium2 kernel — 8-core tensor-parallel SPMD.

Layout strategy: every on-chip tensor is kept in transposed [feature, batch]
layout so the TensorEngine contraction dim is always on partitions and no
on-chip transposes are needed. Host numpy does all transposes / casts /
shard slicing.

Per core c (of 8):
  phase 1: QT/KT [512,2048], V [2048,512] (4 local heads), g1X (gate W1
           X-part, gh-shard) — all from X^T streamed in batch-quarters.
  phase 2: per head: S^T = K^T@Q^T per j-tile, Exp(scale*s + mask_bias) on
           ACT, diagonal zeroed with (1-I) mult, denominator via all-ones
           lhsT matmul (broadcast for free), O^T = V@P^T, normalize.
           AllGather O^T per head (overlaps next head's compute).
  phase 3: cross^T[hid-shard] = Wo[:, shard]^T @ OT_full (column-parallel,
           no reduce needed), k-grouped so matmuls start as AG chunks land.
  phase 4: g1C partials -> ReduceScatter(gh) -> gelu -> AllGather(g^T) ->
           logits[hid-shard] = gW2[:, shard]^T @ gT_full -> sigmoid ->
           out^T = X^T_shard + gate * cross^T.
Host: concat 8 [512,2048] shards, transpose -> [2048,4096] f32.
"""

import numpy as np
import ml_dtypes

import concourse.bass as bass
import concourse.mybir as mybir
import concourse.tile as tile
from concourse import bacc
from concourse import bass_utils

BF16 = mybir.dt.bfloat16
F32 = mybir.dt.float32

B = 2048
HID = 4096
NH = 32
HD = 128
GH = 1024
NC_ = 8
HPC = NH // NC_          # heads per core = 4
HS = HID // NC_          # hid shard = 512
GS = GH // NC_           # gate-hidden shard = 128
SCALE = 1.0 / float(np.sqrt(HD))

KT_TILES = HID // 128    # 32 k-tiles over the 4096 contraction
JT = B // 128            # 16 j-tiles over keys
IC = B // 512            # 4 i-chunks of 512 over batch

# CoreSim doesn't implement Gelu; debug_sim swaps in Tanh.
GELU_FUNC = mybir.ActivationFunctionType.Gelu
# 1..4: truncate the program after this phase (device-fault bisection).
PHASE_LIMIT = 4


def _build_program():
    nc = bacc.Bacc(
        "TRN2",
        target_bir_lowering=False,
        debug=False,
        enable_asserts=False,
        num_devices=NC_,
    )

    # ---- I/O declarations (per-core shapes) ----
    xt_bf = nc.dram_tensor("xt_bf", [HID, B], BF16, kind="ExternalInput").ap()
    xts_f32 = nc.dram_tensor("xts_f32", [HS, B], F32, kind="ExternalInput").ap()
    wq_d = nc.dram_tensor("wq", [HID, HS], BF16, kind="ExternalInput").ap()
    wk_d = nc.dram_tensor("wk", [HID, HS], BF16, kind="ExternalInput").ap()
    wv_d = nc.dram_tensor("wv", [HID, HS], BF16, kind="ExternalInput").ap()
    wo_d = nc.dram_tensor("wo", [HID, HS], BF16, kind="ExternalInput").ap()
    gw1x_d = nc.dram_tensor("gw1x", [HID, GS], BF16, kind="ExternalInput").ap()
    gw1c_d = nc.dram_tensor("gw1c", [HS, GH], BF16, kind="ExternalInput").ap()
    gw2_d = nc.dram_tensor("gw2", [GH, HS], BF16, kind="ExternalInput").ap()
    gb1_d = nc.dram_tensor("gb1", [GS, 1], F32, kind="ExternalInput").ap()
    gb2_d = nc.dram_tensor("gb2", [128, 4], F32, kind="ExternalInput").ap()
    maskb_d = nc.dram_tensor("maskb", [128, JT], F32, kind="ExternalInput").ap()
    diagm_d = nc.dram_tensor("diagm", [128, 128], BF16, kind="ExternalInput").ap()
    out_d = nc.dram_tensor("out", [HS, B], F32, kind="ExternalOutput").ap()

    groups = [list(range(NC_))]

    with tile.TileContext(nc) as tc:
        with (
            tc.tile_pool(name="persist", bufs=1) as persist,
            tc.tile_pool(name="psum", bufs=1, space="PSUM") as psum,
            tc.tile_pool(name="dram", bufs=1, space="DRAM") as dram,
        ):
            # ---------- persistent SBUF ----------
            qt_sb = persist.tile([128, HPC, B], BF16)     # [d, head, i] 2MB
            kt_sb = persist.tile([128, HPC, B], BF16)     # 2MB
            v_sb = persist.tile([128, JT, HS], BF16)      # [j_in, j_tile, hd] 2MB
            g1x_sb = persist.tile([128, B], F32)          # gate W1 X-part 1MB
            maskb_sb = persist.tile([128, JT], F32)
            diagm_sb = persist.tile([128, 128], BF16)
            ones_sb = persist.tile([128, 128], BF16)
            gb1_sb = persist.tile([GS, 1], F32)
            gb2_sb = persist.tile([128, 4], F32)
            cross_bf = persist.tile([128, 4, B], BF16)    # cross^T hid-shard 2MB

            nc.sync.dma_start(out=maskb_sb, in_=maskb_d)
            nc.sync.dma_start(out=diagm_sb, in_=diagm_d)
            nc.sync.dma_start(out=gb1_sb, in_=gb1_d)
            nc.sync.dma_start(out=gb2_sb, in_=gb2_d)
            nc.vector.memset(ones_sb, 1.0)

            # ---------- DRAM bounce buffers for collectives ----------
            ag_in = dram.tile([HPC, 128, B], BF16)        # per-head O^T AG inputs
            ag_out = []
            for h in range(HPC):
                t_ag = dram.tile(
                    [NC_ * 128, B], BF16, addr_space="Shared", name=f"ag_out{h}"
                )
                ag_out.append(t_ag)
            rs_in = dram.tile([GH, B], BF16)
            rs_out = dram.tile([GS, B], BF16)
            ag2_in = dram.tile([GS, B], BF16)
            ag2_out = dram.tile([GH, B], BF16, addr_space="Shared")

            # =====================================================
            # Phase 1: projections, streamed in batch-quarters
            # =====================================================
            with tc.tile_pool(name="p1", bufs=1) as p1:
                gw1x_sb = p1.tile([128, KT_TILES, GS], BF16, tag="gw1x", bufs=1)
                nc.sync.dma_start(
                    out=gw1x_sb, in_=gw1x_d.rearrange("(t p) m -> p t m", p=128)
                )
                for q in range(IC):  # 4 quarters of 512 batch elems
                    isl = slice(q * 512, (q + 1) * 512)
                    xt_q = p1.tile([128, KT_TILES, 512], BF16, tag="xt", bufs=2)
                    nc.sync.dma_start(
                        out=xt_q,
                        in_=xt_bf[:, isl].rearrange("(t p) i -> p t i", p=128),
                    )
                    def load_w_halves(wd, nm):
                        halves = []
                        for hh in range(2):
                            w_sb = p1.tile([128, KT_TILES // 2, HS], BF16,
                                           tag="w", bufs=3, name=nm + str(hh))
                            nc.sync.dma_start(
                                out=w_sb,
                                in_=wd[hh * 2048:(hh + 1) * 2048, :].rearrange(
                                    "(t p) m -> p t m", p=128
                                ),
                            )
                            halves.append(w_sb)
                        return halves

                    def w_slice(halves, k, msl):
                        return halves[k // 16][:, k % 16, msl]

                    for wd, dst, nm in ((wq_d, qt_sb, "wq"), (wk_d, kt_sb, "wk")):
                        wh = load_w_halves(wd, nm)
                        for m in range(4):
                            ps = psum.tile([128, 512], F32, tag="mm", bufs=3,
                                           name="ps_pr")
                            for k in range(KT_TILES):
                                nc.tensor.matmul(
                                    ps,
                                    lhsT=w_slice(wh, k,
                                                 slice(m * 128, (m + 1) * 128)),
                                    rhs=xt_q[:, k, :],
                                    start=(k == 0),
                                    stop=(k == KT_TILES - 1),
                                )
                            nc.scalar.copy(dst[:, m, isl], ps)
                    # V in natural [j, d] layout: lhsT = X^T tiles
                    wvh = load_w_halves(wv_d, "wv")
                    for it in range(4):  # 4 i-tiles of 128 in this quarter
                        ps = psum.tile([128, 512], F32, tag="mm", bufs=3,
                                       name="ps_v")
                        for k in range(KT_TILES):
                            nc.tensor.matmul(
                                ps,
                                lhsT=xt_q[:, k, it * 128:(it + 1) * 128],
                                rhs=w_slice(wvh, k, slice(0, HS)),
                                start=(k == 0),
                                stop=(k == KT_TILES - 1),
                            )
                        nc.scalar.copy(v_sb[:, q * 4 + it, :], ps)
                    # gate W1 X-part (gh-shard output)
                    ps = psum.tile([128, 512], F32, tag="mm", bufs=3, name="ps_g1x")
                    for k in range(KT_TILES):
                        nc.tensor.matmul(
                            ps,
                            lhsT=gw1x_sb[:, k, :],
                            rhs=xt_q[:, k, :],
                            start=(k == 0),
                            stop=(k == KT_TILES - 1),
                        )
                    nc.vector.tensor_copy(g1x_sb[:, isl], ps)

            if PHASE_LIMIT == 1:
                with tc.tile_pool(name="dbg", bufs=1) as dbg:
                    for m in range(4):
                        t_f = dbg.tile([128, B], F32, tag="dbg", bufs=2)
                        nc.vector.tensor_copy(t_f, qt_sb[:, m, :])
                        nc.sync.dma_start(
                            out=out_d[m * 128:(m + 1) * 128, :], in_=t_f
                        )
            # =====================================================
            # Phase 2: attention per head, AllGather O^T per head
            # =====================================================
            with tc.tile_pool(name="p2", bufs=1) as p2:
                for h in range(HPC):
                    for half in range(2):
                        hsl0 = half * 1024
                        den_ps = psum.tile([128, 1024], F32, tag="den", bufs=1)
                        ot_ps = psum.tile([128, 1024], F32, tag="ot", bufs=1)
                        pt = p2.tile([128, JT, 1024], BF16, tag="pt", bufs=2)
                        for j in range(JT):
                            for ic in range(2):
                                st = psum.tile([128, 512], F32, tag="mm", bufs=3,
                                               name="st")
                                nc.tensor.matmul(
                                    st,
                                    lhsT=kt_sb[:, h, j * 128:(j + 1) * 128],
                                    rhs=qt_sb[:, h, hsl0 + ic * 512:
                                              hsl0 + (ic + 1) * 512],
                                    start=True,
                                    stop=True,
                                )
                                nc.scalar.activation(
                                    pt[:, j, ic * 512:(ic + 1) * 512],
                                    st,
                                    mybir.ActivationFunctionType.Exp,
                                    bias=maskb_sb[:, j:j + 1],
                                    scale=SCALE,
                                )
                            # zero the self-attention diagonal block
                            if j * 128 // 1024 == half:
                                c0 = j * 128 - hsl0
                                nc.vector.tensor_mul(
                                    pt[:, j, c0:c0 + 128],
                                    pt[:, j, c0:c0 + 128],
                                    diagm_sb,
                                )
                        for j in range(JT):
                            for ic in range(2):
                                csl = slice(ic * 512, (ic + 1) * 512)
                                nc.tensor.matmul(
                                    den_ps[:, csl],
                                    lhsT=ones_sb,
                                    rhs=pt[:, j, csl],
                                    start=(j == 0),
                                    stop=(j == JT - 1),
                                )
                                nc.tensor.matmul(
                                    ot_ps[:, csl],
                                    lhsT=v_sb[:, j, h * 128:(h + 1) * 128],
                                    rhs=pt[:, j, csl],
                                    start=(j == 0),
                                    stop=(j == JT - 1),
                                )
                        rec = p2.tile([128, 1024], F32, tag="rec", bufs=2)
                        nc.vector.reciprocal(rec, den_ps)
                        otc = p2.tile([128, 1024], BF16, tag="otc", bufs=2)
                        nc.vector.tensor_mul(otc, ot_ps, rec)
                        nc.sync.dma_start(
                            out=ag_in[h, :, hsl0:hsl0 + 1024], in_=otc
                        )
                    nc.gpsimd.collective_compute(
                        "AllGather",
                        mybir.AluOpType.bypass,
                        replica_groups=groups,
                        ins=[ag_in[h].opt()],
                        outs=[ag_out[h].opt()],
                    )

            if PHASE_LIMIT == 2:
                with tc.tile_pool(name="dbg", bufs=1) as dbg:
                    otg0 = dbg.tile([128, 4, B], BF16, tag="dbgb", bufs=1)
                    nc.sync.dma_start(
                        out=otg0,
                        in_=ag_out[0][:512, :].rearrange("(r p) i -> p r i", p=128),
                    )
                    for m in range(4):
                        t_f = dbg.tile([128, B], F32, tag="dbg", bufs=2)
                        nc.vector.tensor_copy(t_f, otg0[:, m, :])
                        nc.sync.dma_start(
                            out=out_d[m * 128:(m + 1) * 128, :], in_=t_f
                        )
            # =====================================================
            # Phase 3: cross^T[hid-shard] = Wo_cs^T @ OT_full
            # k-grouped by AG chunk so compute starts as chunks land
            # =====================================================
            with tc.tile_pool(name="p3", bufs=1) as p3:
                cross_acc = p3.tile([128, 4, B], F32, tag="cacc", bufs=1)
                for t in range(HPC):  # 4 k-groups == 4 AG chunks
                    wo_t = p3.tile([128, NC_, HS], BF16, tag="wo", bufs=2)
                    nc.sync.dma_start(
                        out=wo_t,
                        in_=wo_d[t * 1024:(t + 1) * 1024, :].rearrange(
                            "(r p) m -> p r m", p=128
                        ),
                    )
                    otg = p3.tile([128, NC_, B], BF16, tag="otg", bufs=2)
                    nc.sync.dma_start(
                        out=otg,
                        in_=ag_out[t].rearrange("(r p) i -> p r i", p=128),
                    )
                    for m in range(4):
                        for ic in range(IC):
                            csl = slice(ic * 512, (ic + 1) * 512)
                            ps = psum.tile([128, 512], F32, tag="mm", bufs=3,
                                           name="ps_wo")
                            for r in range(NC_):
                                nc.tensor.matmul(
                                    ps,
                                    lhsT=wo_t[:, r, m * 128:(m + 1) * 128],
                                    rhs=otg[:, r, csl],
                                    start=(r == 0),
                                    stop=(r == NC_ - 1),
                                )
                            if t == 0:
                                nc.vector.tensor_copy(cross_acc[:, m, csl], ps)
                            else:
                                nc.vector.tensor_add(
                                    cross_acc[:, m, csl], cross_acc[:, m, csl], ps
                                )
                for m in range(4):
                    nc.vector.tensor_copy(cross_bf[:, m, :], cross_acc[:, m, :])

                if PHASE_LIMIT == 3:
                    for m in range(4):
                        nc.sync.dma_start(
                            out=out_d[m * 128:(m + 1) * 128, :],
                            in_=cross_acc[:, m, :],
                        )

            # =====================================================
            # Phase 4: gate MLP + final output
            # =====================================================
            with tc.tile_pool(name="p4", bufs=1) as p4:
                gw1c_sb = p4.tile([128, 4, GH], BF16, tag="gw1c", bufs=1)
                nc.sync.dma_start(
                    out=gw1c_sb, in_=gw1c_d.rearrange("(t p) m -> p t m", p=128)
                )
                # g1C partials over local hid-shard of cross
                for m in range(NC_):  # 8 gh-tiles
                    for ic in range(IC):
                        csl = slice(ic * 512, (ic + 1) * 512)
                        ps = psum.tile([128, 512], F32, tag="mm", bufs=3,
                                       name="ps_g1c")
                        for r in range(4):
                            nc.tensor.matmul(
                                ps,
                                lhsT=gw1c_sb[:, r, m * 128:(m + 1) * 128],
                                rhs=cross_bf[:, r, csl],
                                start=(r == 0),
                                stop=(r == 3),
                            )
                        g1c_ch = p4.tile([128, 512], BF16, tag="g1cch", bufs=4)
                        nc.scalar.copy(g1c_ch, ps)
                        nc.sync.dma_start(
                            out=rs_in[m * 128:(m + 1) * 128, csl], in_=g1c_ch
                        )
                nc.gpsimd.collective_compute(
                    "ReduceScatter",
                    mybir.AluOpType.add,
                    replica_groups=groups,
                    ins=[rs_in.opt()],
                    outs=[rs_out.opt()],
                )
                g1c_sb = p4.tile([128, B], BF16, tag="g1c", bufs=1)
                nc.sync.dma_start(out=g1c_sb, in_=rs_out)
                gsum = p4.tile([128, B], F32, tag="gsum", bufs=1)
                nc.vector.tensor_add(gsum, g1x_sb, g1c_sb)
                gt_sb = p4.tile([128, B], BF16, tag="gt", bufs=1)
                nc.scalar.activation(
                    gt_sb, gsum, GELU_FUNC,
                    bias=gb1_sb, scale=1.0,
                )
                nc.sync.dma_start(out=ag2_in, in_=gt_sb)
                nc.gpsimd.collective_compute(
                    "AllGather",
                    mybir.AluOpType.bypass,
                    replica_groups=groups,
                    ins=[ag2_in.opt()],
                    outs=[ag2_out.opt()],
                )
                gtf_sb = p4.tile([128, NC_, B], BF16, tag="gtf", bufs=1)
                nc.sync.dma_start(
                    out=gtf_sb, in_=ag2_out.rearrange("(r p) i -> p r i", p=128)
                )
                gw2_sb = p4.tile([128, NC_, HS], BF16, tag="gw2", bufs=1)
                nc.sync.dma_start(
                    out=gw2_sb, in_=gw2_d.rearrange("(t p) m -> p t m", p=128)
                )
                xts_sb = p4.tile([128, 4, B], F32, tag="xts", bufs=1)
                nc.sync.dma_start(
                    out=xts_sb, in_=xts_f32.rearrange("(t p) i -> p t i", p=128)
                )
                gate_sb = p4.tile([128, 4, B], BF16, tag="gate", bufs=1)
                for m in range(4):
                    for ic in range(IC):
                        csl = slice(ic * 512, (ic + 1) * 512)
                        ps = psum.tile([128, 512], F32, tag="mm", bufs=3,
                                       name="ps_gw2")
                        for r in range(NC_):
                            nc.tensor.matmul(
                                ps,
                                lhsT=gw2_sb[:, r, m * 128:(m + 1) * 128],
                                rhs=gtf_sb[:, r, csl],
                                start=(r == 0),
                                stop=(r == NC_ - 1),
                            )
                        nc.scalar.activation(
                            gate_sb[:, m, csl], ps,
                            mybir.ActivationFunctionType.Sigmoid,
                            bias=gb2_sb[:, m:m + 1], scale=1.0,
                        )
                for m in range(4):
                    outt = p4.tile([128, B], F32, tag="outt", bufs=2)
                    nc.vector.tensor_mul(outt, gate_sb[:, m, :], cross_bf[:, m, :])
                    nc.vector.tensor_add(outt, outt, xts_sb[:, m, :])
                    nc.sync.dma_start(
                        out=out_d[m * 128:(m + 1) * 128, :], in_=outt
                    )

    nc.compile()
    return nc


def _make_in_maps(inputs):
    f32 = np.float32
    bf = ml_dtypes.bfloat16
    X = np.asarray(inputs["hidden_states"], dtype=f32)
    mask = np.asarray(inputs["attention_mask"])
    Wq = np.asarray(inputs["Wq"], dtype=f32)
    Wk = np.asarray(inputs["Wk"], dtype=f32)
    Wv = np.asarray(inputs["Wv"], dtype=f32)
    Wo = np.asarray(inputs["Wo"], dtype=f32)
    gW1 = np.asarray(inputs["gW1"], dtype=f32)
    gb1 = np.asarray(inputs["gb1"], dtype=f32)
    gW2 = np.asarray(inputs["gW2"], dtype=f32)
    gb2 = np.asarray(inputs["gb2"], dtype=f32)

    XT = np.ascontiguousarray(X.T)                       # [4096, 2048]
    XT_bf = XT.astype(bf)
    # Wo row permutation to match per-head AllGather chunk assembly:
    # OT_full row (t*1024 + r*128 + d) holds global head (4r+t), dim d.
    perm = np.empty(HID, dtype=np.int64)
    for t in range(HPC):
        for r in range(NC_):
            g = 4 * r + t
            perm[t * 1024 + r * 128:t * 1024 + (r + 1) * 128] = np.arange(
                g * 128, (g + 1) * 128
            )
    Wo_p = Wo[perm]
    maskb = np.where(mask, 0.0, -1e30).astype(f32)       # [2048]
    maskb_t = np.ascontiguousarray(maskb.reshape(JT, 128).T)  # [128, 16]
    diagm = (1.0 - np.eye(128, dtype=f32)).astype(bf)

    in_maps = []
    for c in range(NC_):
        hsl = slice(c * HS, (c + 1) * HS)
        gsl = slice(c * GS, (c + 1) * GS)
        in_maps.append({
            "xt_bf": XT_bf,
            "xts_f32": np.ascontiguousarray(XT[hsl]),
            "wq": np.ascontiguousarray(Wq[:, hsl].astype(bf)),
            "wk": np.ascontiguousarray(Wk[:, hsl].astype(bf)),
            "wv": np.ascontiguousarray(Wv[:, hsl].astype(bf)),
            "wo": np.ascontiguousarray(Wo_p[:, hsl].astype(bf)),
            "gw1x": np.ascontiguousarray(gW1[:HID, gsl].astype(bf)),
            "gw1c": np.ascontiguousarray(gW1[HID + c * HS:HID + (c + 1) * HS].astype(bf)),
            "gw2": np.ascontiguousarray(gW2[:, hsl].astype(bf)),
            "gb1": np.ascontiguousarray(gb1[gsl].reshape(GS, 1)),
            "gb2": np.ascontiguousarray(gb2[hsl].reshape(4, 128).T),
            "maskb": maskb_t,
            "diagm": diagm,
        })
    return in_maps


_NC_CACHE = None


def _run(inputs, trace=False):
    global _NC_CACHE
    if _NC_CACHE is None:
        _NC_CACHE = _build_program()
    nc = _NC_CACHE
    in_maps = _make_in_maps(inputs)
    res = bass_utils.run_bass_kernel_spmd(
        nc, in_maps, core_ids=list(range(NC_)), trace=trace
    )
    shards = [np.asarray(res.results[c]["out"], dtype=np.float32)
              for c in range(NC_)]
    out = np.ascontiguousarray(np.concatenate(shards, axis=0).T)
    return out, res


def kernel(**inputs) -> np.ndarray:
    out, _ = _run(inputs, trace=False)
    return out
